# revision 14
# baseline (speedup 1.0000x reference)
"""StyleGAN2-style modulated 3x3 conv layer on 8 TRN2 NeuronCores.

Math (per sample b):
    style = latent @ (fc_weight * LAT**-0.5).T + fc_bias            [CIN]
    w     = weight * style[None,:,None,None]                        [COUT,CIN,3,3]
    w     = w * rsqrt(sum(w*w, (1,2,3)) + EPS) * w_mul_conv
    out   = lrelu(conv2d(x, w, pad=1) + bias, 0.2)

Folded form (weights stay shared across the batch; style moves to the input,
demod to the output):
    out[o] = lrelu(d[o] * conv2d(x * style, weight)[o] + bias[o])
    d[o]   = w_mul_conv * rsqrt(sum_i style_i^2 * ssq[o,i])   (ssq host-precomp)

Conv algorithm: width-direction Winograd F(4,3) with Cook-Toom points
{0, 1, -1, 1/2, -2} (chosen for bf16 accuracy; rel err ~8e-3 vs 2e-2 budget).
Height stays a direct 3-tap accumulation via row shifts.

The input transform B^T is style-independent (B^T is spatial, style is
per-channel, they commute), so it runs on HOST like the weight G-transform:
the device receives 6 pre-transformed V planes per ci tile, applies the
per-channel style scale (one 4x-mode tensor_scalar per ci), then runs
4co x 2half x 6g PSUM groups of 12 bf16 matmuls (4ci x 3kh row-shifts) of
512 cols each = 294,912 PE columns (~123us @ 2.4GHz; the F(2,3) baseline
streamed 393,216).  ACT evacuates each PSUM bank with the demod scale
(m1 also adds the conv bias so every output combo inherits it exactly once),
DVE does the A^T output combine in fp32 (in-place chains: y0=c, y1=p, y2=a,
y3=b), and lrelu(x) = 0.2x + 0.8*relu(x) splits across ACT (strided
relu write into the interleaved yt) and DVE (one stt per phase).

Sharding: data-parallel over batch B=8, one sample per core; weights
replicated.  reps>1 replicates the per-sample body with double-buffered
tiles so consecutive reps pipeline (used for slope-timing on HW): style is
computed one rep ahead, and each half-image's V block (DMA + style scale)
is emitted one half early — during the previous half's co=1 block — so the
first matmuls of a half never wait behind the DVE combine queue.
"""

from fractions import Fraction as Fr

import numpy as np
import ml_dtypes

B, CIN, COUT, K, LAT, H, W = 8, 512, 512, 3, 512, 64, 64
EPS = 1e-8
W_MUL_FC = LAT**-0.5
W_MUL_CONV = (2.0**0.5) * (CIN * K * K) ** -0.5

P = 128
CI_T = CIN // P  # 4 input-channel tiles
CO_T = COUT // P  # 4 output-channel tiles
LA_T = LAT // P  # 4 latent tiles
M_W = 4  # Winograd F(M_W, 3) along width
NG = M_W + 2  # 6 V planes
NJ = W // M_W  # 16 tiles per row
HALF = 2  # image processed in two 32-row halves
HR = H // HALF + 2  # V rows per half (32 outputs + 2 pad overlap)
VC = NG * HR * NJ  # 3264 elements per (ci, half) V block
NMAX = 512  # PSUM bank columns = 32 rows x 16 j
POINTS = (0, 1, -1, Fr(1, 2), -2)

_COMPILED = {}


def _cook_toom(points, m=M_W, r=3):
    """Exact Cook-Toom transforms: y = A^T [(G w) * (B^T d)]."""
    n = m + r - 1
    a = [Fr(p) for p in points]
    C = [[a[i] ** j for j in range(n)] for i in range(n - 1)] + [
        [Fr(0)] * (n - 1) + [Fr(1)]
    ]
    Aug = [row[:] + [Fr(1) if k == i else Fr(0) for k in range(n)]
           for i, row in enumerate(C)]
    for col in range(n):
        piv = next(rr for rr in range(col, n) if Aug[rr][col] != 0)
        Aug[col], Aug[piv] = Aug[piv], Aug[col]
        pv = Aug[col][col]
        Aug[col] = [v / pv for v in Aug[col]]
        for rr in range(n):
            if rr != col and Aug[rr][col] != 0:
                f = Aug[rr][col]
                Aug[rr] = [v - f * w for v, w in zip(Aug[rr], Aug[col])]
    Cinv = [row[n:] for row in Aug]
    f = [Fr(1)] * n
    for i in range(n - 1):
        for j in range(n - 1):
            if i != j:
                f[i] *= a[i] - a[j]
    BT = [[f[i] * Cinv[j][i] for j in range(n)] for i in range(n)]
    G = [[(a[i] ** j) / f[i] for j in range(r)] for i in range(n - 1)] + [
        [Fr(0)] * (r - 1) + [Fr(1)]
    ]
    AT = [[a[i] ** k for i in range(n - 1)] + [Fr(0)] for k in range(m)]
    AT[m - 1][n - 1] = Fr(1)
    to_np = lambda Mx: np.array([[float(v) for v in row] for row in Mx], np.float64)
    return to_np(AT), to_np(G), to_np(BT)


AT_W, G_W, BT_W = _cook_toom(POINTS)
# AT_W == [[1,1,1,1,1,0],[0,1,-1,.5,-2,0],[0,1,1,.25,4,0],[0,1,-1,.125,-8,1]]
# (the device combine below hardcodes this structure)
assert np.allclose(AT_W, [[1, 1, 1, 1, 1, 0], [0, 1, -1, 0.5, -2, 0],
                          [0, 1, 1, 0.25, 4, 0], [0, 1, -1, 0.125, -8, 1]])


def _build_nc(reps=1):
    import concourse.bass as bass
    import concourse.mybir as mybir
    from concourse import bacc
    from concourse.tile import TileContext

    fp32 = mybir.dt.float32
    bf16 = mybir.dt.bfloat16
    AF = mybir.ActivationFunctionType
    ALU = mybir.AluOpType

    nc = bacc.Bacc("TRN2", target_bir_lowering=False, debug=False)

    # style/demod matvec operands are bf16: keeps the PE stream uniformly
    # bf16 (no fp32 LDWEIGHTS, which has no fast-weight-load path, and no
    # PE dtype-mode switches mid-rep); fp32 accumulation in PSUM as before.
    vr_d = nc.dram_tensor("vr", [HALF, CI_T, P, VC], bf16, kind="ExternalInput")
    lat_d = nc.dram_tensor("lat", [LAT], bf16, kind="ExternalInput")
    wt_d = nc.dram_tensor("wt", [CI_T, P, CO_T * 3 * NG * P], bf16, kind="ExternalInput")
    fct_d = nc.dram_tensor("fct", [LA_T, P, CIN], bf16, kind="ExternalInput")
    ssq_d = nc.dram_tensor("ssq", [CI_T, P, COUT], bf16, kind="ExternalInput")
    fcb_d = nc.dram_tensor("fcb", [P, CI_T], fp32, kind="ExternalInput")
    cb_d = nc.dram_tensor("cbias", [P, CO_T], fp32, kind="ExternalInput")
    out_d = nc.dram_tensor("out", [COUT, H, W], fp32, kind="ExternalOutput")

    inv_wmc2 = 1.0 / (W_MUL_CONV * W_MUL_CONV)

    with TileContext(nc) as tc, tc.tile_pool(name="persist", bufs=1) as persist:
        def tile0(shape, dtype, name):
            return persist.tile(shape, dtype, tag=name, name=name)

        # --- constants / weights: DMA'd once ---
        fct = [tile0([P, CIN], bf16, f"fct{i}") for i in range(LA_T)]
        fcb = tile0([P, CI_T], fp32, "fcb")
        cb = tile0([P, CO_T], fp32, "cb")
        ssq = [tile0([P, COUT], bf16, f"ssq{i}") for i in range(CI_T)]
        wsb = [tile0([P, CO_T * 3 * NG * P], bf16, f"wsb{i}") for i in range(CI_T)]

        for l in range(LA_T):
            nc.sync.dma_start(fct[l][:], fct_d[l])
        nc.sync.dma_start(fcb[:], fcb_d[:])
        nc.sync.dma_start(cb[:], cb_d[:])
        for ci in range(CI_T):
            nc.sync.dma_start(ssq[ci][:], ssq_d[ci])

        with (
            tc.tile_pool(name="vpool", bufs=2) as vpool,
            tc.tile_pool(name="mpool", bufs=2) as mpool,
            tc.tile_pool(name="apool", bufs=2) as apool,
            tc.tile_pool(name="ytpool", bufs=2) as ytpool,
            tc.tile_pool(name="spool", bufs=2) as spool,
            tc.tile_pool(name="dtmp", bufs=2) as dpool,
            tc.tile_pool(name="pconv", bufs=8, space="PSUM") as pconv,
        ):
            def emit_style(rep):
                """Style + demod-scale for `rep` (spool/dpool rings, bufs=2)."""
                latsb = spool.tile([P, LA_T], bf16, tag="latsb", name=f"latsb_{rep}")
                nc.sync.dma_start(latsb[:], lat_d[:].rearrange("(l p) -> p l", p=P))
                style = [
                    spool.tile([P, 1], fp32, tag=f"style{i}", name=f"style{i}_{rep}")
                    for i in range(CI_T)
                ]
                style2 = [
                    spool.tile([P, 1], bf16, tag=f"style2_{i}", name=f"style2_{i}_{rep}")
                    for i in range(CI_T)
                ]
                dscale = [
                    spool.tile([P, 1], fp32, tag=f"dscale{i}", name=f"dscale{i}_{rep}")
                    for i in range(CO_T)
                ]
                for ci in range(CI_T):
                    ps = pconv.tile([P, NMAX], fp32, tag="ps_conv",
                                    name=f"ps_st{ci}_{rep}")[:, :1]
                    for l in range(LA_T):
                        nc.tensor.matmul(
                            ps[:],
                            lhsT=fct[l][:, ci * P : (ci + 1) * P],
                            rhs=latsb[:, l : l + 1],
                            start=(l == 0),
                            stop=(l == LA_T - 1),
                        )
                    nc.scalar.activation(
                        style[ci][:], ps[:], AF.Identity,
                        bias=fcb[:, ci : ci + 1], scale=W_MUL_FC,
                    )
                    nc.scalar.activation(
                        style2[ci][:], ps[:], AF.Square,
                        bias=fcb[:, ci : ci + 1], scale=W_MUL_FC,
                    )
                for co in range(CO_T):
                    ps = pconv.tile([P, NMAX], fp32, tag="ps_conv",
                                    name=f"ps_d{co}_{rep}")[:, :1]
                    for ci in range(CI_T):
                        nc.tensor.matmul(
                            ps[:],
                            lhsT=ssq[ci][:, co * P : (co + 1) * P],
                            rhs=style2[ci][:],
                            start=(ci == 0),
                            stop=(ci == CI_T - 1),
                        )
                    sarg = dpool.tile([P, 1], fp32, tag="sarg", name=f"sarg{co}_{rep}")
                    sq = dpool.tile([P, 1], fp32, tag="sq", name=f"sq{co}_{rep}")
                    y0 = dpool.tile([P, 1], fp32, tag="y0", name=f"y0_{co}_{rep}")
                    u = dpool.tile([P, 1], fp32, tag="u", name=f"u{co}_{rep}")
                    v = dpool.tile([P, 1], fp32, tag="v", name=f"v{co}_{rep}")
                    # sarg = s / wmc^2 ; d = 1/sqrt(sarg). EPS=1e-8 vs s~O(1e3)
                    # is ~1e-11 relative - dropped.
                    nc.scalar.activation(
                        sarg[:], ps[:], AF.Identity, bias=0.0, scale=inv_wmc2
                    )
                    nc.scalar.activation(
                        sq[:], ps[:], AF.Sqrt, bias=0.0, scale=inv_wmc2
                    )
                    nc.vector.reciprocal(y0[:], sq[:])
                    # Newton: y1 = y0*(1.5 - 0.5*sarg*y0^2)
                    nc.vector.tensor_mul(u[:], y0[:], y0[:])
                    nc.vector.tensor_mul(v[:], u[:], sarg[:])
                    nc.vector.tensor_scalar(
                        v[:], v[:], -0.5, 1.5, op0=ALU.mult, op1=ALU.add
                    )
                    nc.vector.tensor_mul(dscale[co][:], y0[:], v[:])
                return style, dscale

            def emit_v(rep, h, style_r):
                """DMA + style-scale the V block for (rep, h)."""
                vsb = [
                    vpool.tile([P, VC], bf16, tag=f"v{ci}", name=f"v{ci}_{h}_{rep}")
                    for ci in range(CI_T)
                ]
                for ci in range(CI_T):
                    nc.sync.dma_start(vsb[ci][:], vr_d[h, ci])
                for ci in range(CI_T):
                    nc.vector.tensor_scalar_mul(vsb[ci][:], vsb[ci][:], style_r[ci][:])
                return vsb

            style, dscale = emit_style(0)
            style_next = dscale_next = None
            # co=0 weight chunk first: first conv matmuls need it
            for ci in range(CI_T):
                nc.sync.dma_start(wsb[ci][:, : 3 * NG * P], wt_d[ci, :, : 3 * NG * P])
            vsb = emit_v(0, 0, style)
            for co in range(1, CO_T):
                for ci in range(CI_T):
                    s = co * 3 * NG * P
                    nc.sync.dma_start(
                        wsb[ci][:, s : s + 3 * NG * P], wt_d[ci, :, s : s + 3 * NG * P]
                    )

            for rep in range(reps):
                for h in range(HALF):
                    vsb_next = None

                    for co in range(CO_T):
                        ps = [
                            pconv.tile([P, NMAX], fp32, tag="ps_conv",
                                       name=f"pc{co}_{h}_{g}_{rep}")
                            for g in range(NG)
                        ]
                        for g in range(NG):
                            idx = 0
                            for ci in range(CI_T):
                                for kh in range(3):
                                    nc.tensor.matmul(
                                        ps[g][:],
                                        lhsT=wsb[ci][
                                            :,
                                            ((co * 3 + kh) * NG + g) * P
                                            : ((co * 3 + kh) * NG + g + 1) * P,
                                        ],
                                        rhs=vsb[ci][
                                            :, g * HR * NJ + kh * NJ
                                            : g * HR * NJ + kh * NJ + NMAX
                                        ],
                                        start=(idx == 0),
                                        stop=(idx == 3 * CI_T - 1),
                                    )
                                    idx += 1
                        # ACT evac with demod scale; m1 carries the conv bias so
                        # every A^T combo (via a=m1+m2 or b=m1-m2) gets it once.
                        m = []
                        for g in range(NG):
                            mt = mpool.tile([P, NMAX], fp32, tag=f"m{g}",
                                            name=f"m{g}_{co}_{h}_{rep}")
                            nc.scalar.activation(
                                mt[:], ps[g][:], AF.Identity,
                                bias=(cb[:, co : co + 1] if g == 1 else 0.0),
                                scale=dscale[co][:],
                            )
                            m.append(mt)
                        # A^T combine (fp32, in-place chains):
                        #   y0 = (m1+m2) + (m3+m4) + m0          -> c
                        #   y1 = 0.5 m3 + (-2 m4 + (m1-m2))      -> p
                        #   y2 = 0.25 m3 + (4 m4 + (m1+m2))      -> a
                        #   y3 = 0.125 m3 + (-8 m4 + (m1-m2)) + m5 -> b
                        a = apool.tile([P, NMAX], fp32, tag="a", name=f"a_{co}_{h}_{rep}")
                        b = apool.tile([P, NMAX], fp32, tag="b", name=f"b_{co}_{h}_{rep}")
                        c = apool.tile([P, NMAX], fp32, tag="c", name=f"c_{co}_{h}_{rep}")
                        p = apool.tile([P, NMAX], fp32, tag="p", name=f"p_{co}_{h}_{rep}")
                        nc.vector.tensor_add(a[:], m[1][:], m[2][:])
                        nc.vector.tensor_sub(b[:], m[1][:], m[2][:])
                        nc.vector.tensor_add(c[:], m[3][:], m[4][:])
                        nc.vector.tensor_add(c[:], c[:], a[:])
                        nc.vector.tensor_add(c[:], c[:], m[0][:])  # y0
                        nc.vector.scalar_tensor_tensor(
                            p[:], m[4][:], -2.0, b[:], op0=ALU.mult, op1=ALU.add
                        )
                        nc.vector.scalar_tensor_tensor(
                            p[:], m[3][:], 0.5, p[:], op0=ALU.mult, op1=ALU.add
                        )  # y1
                        nc.vector.scalar_tensor_tensor(
                            a[:], m[4][:], 4.0, a[:], op0=ALU.mult, op1=ALU.add
                        )
                        nc.vector.scalar_tensor_tensor(
                            a[:], m[3][:], 0.25, a[:], op0=ALU.mult, op1=ALU.add
                        )  # y2
                        nc.vector.scalar_tensor_tensor(
                            b[:], m[4][:], -8.0, b[:], op0=ALU.mult, op1=ALU.add
                        )
                        nc.vector.scalar_tensor_tensor(
                            b[:], m[3][:], 0.125, b[:], op0=ALU.mult, op1=ALU.add
                        )
                        nc.vector.tensor_add(b[:], b[:], m[5][:])  # y3
                        ys = [c, p, a, b]
                        # lrelu(y) = 0.2 y + 0.8 relu(y): ACT writes relu(0.8 y)
                        # strided into yt, DVE adds 0.2 y in place.
                        yt = ytpool.tile([P, M_W * NMAX], fp32, tag="yt",
                                         name=f"yt_{co}_{h}_{rep}")
                        ytv = yt[:].rearrange("p (r j t) -> p r j t", j=NJ, t=M_W)
                        for t in range(M_W):
                            nc.scalar.activation(
                                ytv[:, :, :, t],
                                ys[t][:].rearrange("p (r j) -> p r j", j=NJ),
                                AF.Relu, bias=0.0, scale=0.8,
                            )
                        for t in range(M_W):
                            nc.vector.scalar_tensor_tensor(
                                ytv[:, :, :, t],
                                ys[t][:].rearrange("p (r j) -> p r j", j=NJ),
                                0.2, ytv[:, :, :, t], op0=ALU.mult, op1=ALU.add,
                            )
                        nc.sync.dma_start(
                            out_d[co * P : (co + 1) * P,
                                  (H // HALF) * h : (H // HALF) * (h + 1), :],
                            yt[:].rearrange("p (r w) -> p r w", w=W),
                        )
                        if h == 0 and co == 0 and rep + 1 < reps:
                            style_next, dscale_next = emit_style(rep + 1)
                        if co == 1:
                            # software-pipeline the next half-image's V block
                            # (DMA + style scale) so the first matmuls of the
                            # next half / next rep never wait on the DVE queue
                            if h == 0:
                                vsb_next = emit_v(rep, 1, style)
                            elif rep + 1 < reps:
                                vsb_next = emit_v(rep + 1, 0, style_next)
                    if vsb_next is not None:
                        vsb = vsb_next
                if rep + 1 < reps:
                    style, dscale = style_next, dscale_next

    nc.compile()
    return nc


def _get_compiled(reps=1):
    if reps not in _COMPILED:
        _COMPILED[reps] = _build_nc(reps)
    return _COMPILED[reps]


def _prep_inputs(x, latent, weight, bias, fc_weight, fc_bias):
    """Host-side layout preprocessing + Winograd transforms (both linear maps
    independent of per-sample style; conv FLOPs stay on device)."""
    bfd = ml_dtypes.bfloat16
    BT32 = BT_W.astype(np.float32)

    # taps d_t[r, j] = xpad[r, 4j+t], t=0..5  -> V[g] = sum_t BT[g,t] d_t
    xp = np.pad(np.asarray(x, np.float32), ((0, 0), (0, 0), (1, 1), (1, 1)))
    taps = np.stack(
        [xp[:, :, :, t : t + 4 * (NJ - 1) + 1 : 4] for t in range(NG)], axis=0
    )  # [6, B, CIN, 66, 16]
    V = np.einsum("gt,tbcrj->gbcrj", BT32, taps, optimize=True)  # [6,B,CIN,66,16]
    # halves: rows 32h .. 32h+33
    vh = np.stack([V[:, :, :, 32 * h : 32 * h + HR, :] for h in range(HALF)], axis=0)
    # [HALF, NG, B, CIN, HR, NJ] -> [B, HALF, CIN(=CI_T*P), NG, HR, NJ] -> flat
    vh = vh.transpose(2, 0, 3, 1, 4, 5).reshape(B, HALF, CI_T, P, VC)
    vr = np.ascontiguousarray(vh).astype(bfd)

    # weights: U[o,i,kh,g] = sum_k G[g,k] w[o,i,kh,k]
    Ukh = np.einsum("gk,oihk->oihg", G_W, weight.astype(np.float64))
    # wt[ci, p, ((co*3+kh)*NG+g)*P + m] = U[co*P+m, ci*P+p, kh, g]
    U6 = Ukh.reshape(CO_T, P, CI_T, P, 3, NG)  # [co, m, ci, p, kh, g]
    wt = np.ascontiguousarray(U6.transpose(2, 3, 0, 4, 5, 1)).reshape(
        CI_T, P, CO_T * 3 * NG * P
    ).astype(bfd)

    fct = np.ascontiguousarray(fc_weight.T).reshape(LA_T, P, CIN).astype(bfd)
    ssq = np.ascontiguousarray(
        (weight.astype(np.float64) ** 2).sum(axis=(2, 3)).T
    ).reshape(CI_T, P, COUT).astype(bfd)
    fcb = np.ascontiguousarray(fc_bias.reshape(CI_T, P).T).astype(np.float32)
    cbv = np.ascontiguousarray(bias.reshape(CO_T, P).T).astype(np.float32)
    lat = np.ascontiguousarray(latent).astype(bfd)

    in_maps = []
    for b in range(B):
        in_maps.append(
            {
                "vr": vr[b],
                "lat": lat[b],
                "wt": wt,
                "fct": fct,
                "ssq": ssq,
                "fcb": fcb,
                "cbias": cbv,
            }
        )
    return in_maps


def kernel(x, latent, weight, bias, fc_weight, fc_bias):
    from concourse.bass_utils import run_bass_kernel_spmd

    x = np.asarray(x, np.float32)
    latent = np.asarray(latent, np.float32)
    weight = np.asarray(weight, np.float32)
    bias = np.asarray(bias, np.float32)
    fc_weight = np.asarray(fc_weight, np.float32)
    fc_bias = np.asarray(fc_bias, np.float32)

    nc = _get_compiled()
    in_maps = _prep_inputs(x, latent, weight, bias, fc_weight, fc_bias)
    res = run_bass_kernel_spmd(nc, in_maps, core_ids=list(range(B)))
    out = np.stack([res.results[b]["out"] for b in range(B)], axis=0)
    return out.astype(np.float32)
